# revision 19
# baseline (speedup 1.0000x reference)
"""Trainium2 Bass kernel for nn_AutoEncoder_48052094108202.

  h = x @ W1 + b1          # [B, H]
  y = h @ W2 + b2          # [B, D]
  out = segmented_softmax(y, segment_ids)   # softmax over contiguous
                                            # feature segments, per row

B=8192, D=4096, H=2048, S=512 segments. Data-parallel over B across 8
NeuronCores (1024 rows/core), weights replicated.

Per-core layout: everything runs transposed (features on SBUF partitions,
batch on the free axis) so no on-device transposes are needed — the host
pre-packs x^T (and un-transposes the output). The segmented softmax is done
entirely on the tensor engine with one-hot matmuls (exact — every product is
1.0 * x):
  seg_sums   s[seg, b] = C_g^T @ e     (C one-hot features->segments)
  recip      r = exp(-ln(s))           (ACT engine; ln+exp share one
                                        activation table, so no reloads)
  broadcast  d[feat, b] = C_g @ r      (one-hot rows)
  out        = e * d                   (DVE)
Matmuls run in bf16 (inputs/weights rounded on host), accumulation in fp32
PSUM; the last 12 k-tiles of the encoder contraction run as 6 fp8-e4m3
DoubleRow pairs (2x PE rate; measured end-to-end rel-err ~1.76e-2 vs the
2e-2 budget). exp() on the ACT engine with the bias folded in.

Segment groups are adaptive: coarse (~5 feature-tiles) early, fine (2) near
the end of the feature axis, so the final reduce->recip->broadcast tail only
blocks the last couple of output tiles. Empty segments get a fake one-hot
entry in their group's first reduce tile so their sums stay positive (their
broadcast columns are all-zero, so outputs are unaffected) — no eps matmuls
and no 1/0.

The batch shard is processed in 2 chunks of 512 columns to fit SBUF.
"""

import os
import sys

import numpy as np

# ---------------------------------------------------------------- constants
B, D, H, S = 8192, 4096, 2048, 512
NCORES = 8
BS = B // NCORES  # 1024 batch rows per core
NB = 2  # chunks per core
BC = BS // NB  # 512 batch rows per chunk
KD = D // 128  # 32 k-tiles over D
KH = H // 128  # 16 k-tiles over H
KF8 = 12  # phase-A k-tiles run in fp8 (the last ones)
NP8 = KF8 // 2  # DoubleRow pairs
KBF = KD - KF8  # bf16 k-tiles in phase A

_WAIT_LIMIT = 1  # walrus CoreV3 accepts 1 sync-wait per instruction


def _import_concourse():
    try:
        import concourse  # noqa: F401
    except ImportError:
        for p in ("/opt/trn_rl_repo", "/root/.axon_site/_ro/trn_rl_repo"):
            if os.path.isdir(p) and p not in sys.path:
                sys.path.insert(0, p)
        import concourse  # noqa: F401


def _split_excess_waits(nc, limit=_WAIT_LIMIT):
    """walrus rejects instructions carrying more than one sync-wait; hoist
    extras onto preceding NOPs on the same engine (same semantics: blocking
    waits on one sequencer, order irrelevant)."""
    import bass_rust

    engines = nc.engines
    for fn in nc.m.functions:
        for bb in fn.blocks:
            insts = bb.instructions
            i = 0
            while i < len(insts):
                inst = insts[i]
                si = inst.sync_info
                waits = list(si.on_wait) if si and si.on_wait else []
                if len(waits) > limit:
                    overflow, keep = waits[:-limit], waits[-limit:]
                    si.on_wait = keep
                    pos = i
                    for j in range(0, len(overflow), limit):
                        nop = engines[inst.engine].nop(
                            nofuse=True, hint="wait_split"
                        ).ins
                        for b2 in fn.blocks:
                            lst = b2.instructions
                            if nop in lst:
                                lst.remove(nop)
                        nop.sync_info = bass_rust.SyncInfo(
                            on_wait=overflow[j : j + limit], on_update=[]
                        )
                        insts.insert(pos, nop)
                        pos += 1
                        i += 1
                i += 1


def _group_limit(k_first):
    """Max k-tile span of a segment group, tapered so the last groups (the
    ones the kernel tail waits on) cover few feature tiles."""
    if k_first >= 26:
        return 2
    if k_first >= 20:
        return 3
    return 5


def _segment_plan(seg):
    """Static plan from the (sorted) segment ids.

    Groups are runs of consecutive segments whose feature span is at most
    _group_limit k-tiles. Returns:
      groups:   list of dicts(base, end, nseg, k_first, k_last)
      red:      list of (k, g) reduce one-hot slots, ordered
      red_by_k: k -> list of (g, slot, is_first, is_last)
      bc:       list of (m, g) bcast one-hot slots, ordered
      m_groups: m -> [g, ...]
      bc_slot:  (m, g) -> slot index
    """
    seg = np.asarray(seg).astype(np.int64)
    assert seg.shape == (D,)
    first_feat = np.full(S, -1, dtype=np.int64)
    last_feat = np.full(S, -1, dtype=np.int64)
    for f in range(D):
        s = seg[f]
        if first_feat[s] < 0:
            first_feat[s] = f
        last_feat[s] = f

    groups = []
    cur = None
    pending_empty = 0  # leading empty segments before any non-empty one
    for s in range(S):
        if first_feat[s] < 0:
            if cur is not None and s + 1 - cur["base"] <= 128:
                cur["end"] = s + 1
            else:
                pending_empty += 1
            continue
        kf, kl = first_feat[s] // 128, last_feat[s] // 128
        if cur is None:
            cur = {"base": s - pending_empty, "end": s + 1, "k_first": kf,
                   "k_last": kl}
            pending_empty = 0
        else:
            nk_last = max(cur["k_last"], kl)
            span = nk_last - cur["k_first"] + 1
            if span > _group_limit(cur["k_first"]) or s + 1 - cur["base"] > 128:
                groups.append(cur)
                cur = {"base": s, "end": s + 1, "k_first": kf, "k_last": kl}
            else:
                cur["end"] = s + 1
                cur["k_last"] = nk_last
    assert cur is not None
    cur["end"] = S  # trailing empty segments ride in the last group
    groups.append(cur)
    for g in groups:
        g["nseg"] = g["end"] - g["base"]
        assert 0 < g["nseg"] <= 128

    red = []
    red_by_k = {k: [] for k in range(KD)}
    for gi, g in enumerate(groups):
        for k in range(g["k_first"], g["k_last"] + 1):
            slot = len(red)
            red.append((k, gi))
            red_by_k[k].append(
                (gi, slot, k == g["k_first"], k == g["k_last"])
            )

    seg_of_tile = [
        (int(seg[m * 128]), int(seg[m * 128 + 127])) for m in range(KD)
    ]
    bc = []
    m_groups = {}
    bc_slot = {}
    for m in range(KD):
        lo, hi = seg_of_tile[m]
        gl = [
            gi
            for gi, g in enumerate(groups)
            if g["base"] <= hi and lo < g["end"]
        ]
        m_groups[m] = gl
        for gi in gl:
            bc_slot[(m, gi)] = len(bc)
            bc.append((m, gi))
    return groups, red, red_by_k, bc, m_groups, bc_slot


def _build_program(seg):
    """Build the (SPMD, per-core) Bass program. Same program on all cores."""
    _import_concourse()
    import concourse.bass as bass
    import concourse.mybir as mybir
    from concourse import tile

    dt = mybir.dt
    AF = mybir.ActivationFunctionType
    DR = mybir.MatmulPerfMode.DoubleRow

    groups, red, red_by_k, bc, m_groups, bc_slot = _segment_plan(seg)
    NR, NBC = len(red), len(bc)

    nc = bass.Bass("TRN2", target_bir_lowering=False, debug=False)

    xtp = nc.dram_tensor("xtp", [NB, 128, KBF, BC], dt.bfloat16, kind="ExternalInput")
    xtp8 = nc.dram_tensor("xtp8", [NB, 128, NP8, 2, BC], dt.float8e4, kind="ExternalInput")
    w1p = nc.dram_tensor("w1p", [KH, 128, KBF, 128], dt.bfloat16, kind="ExternalInput")
    w1p8 = nc.dram_tensor("w1p8", [KH, 128, NP8, 2, 128], dt.float8e4, kind="ExternalInput")
    w2p = nc.dram_tensor("w2p", [KD, 128, KH, 128], dt.bfloat16, kind="ExternalInput")
    b1p = nc.dram_tensor("b1p", [128, KH], dt.float32, kind="ExternalInput")
    b2p = nc.dram_tensor("b2p", [128, KD], dt.float32, kind="ExternalInput")
    # one-hot tiles, partition-major so each loads as a single DMA
    cpp = nc.dram_tensor("cpp", [128, NR, 128], dt.bfloat16, kind="ExternalInput")
    ctpp = nc.dram_tensor("ctpp", [128, NBC, 128], dt.bfloat16, kind="ExternalInput")
    outp = nc.dram_tensor("outp", [KD, 128, BS], dt.float32, kind="ExternalOutput")

    with tile.TileContext(nc) as tc:
        with (
            tc.tile_pool(name="pbig", bufs=1) as pbig,
            tc.tile_pool(name="pw", bufs=5) as pw,
            tc.tile_pool(name="pev", bufs=4) as pev,
            tc.tile_pool(name="pr32", bufs=2) as pr32,
            tc.tile_pool(name="psmall", bufs=1) as psmall,
            tc.tile_pool(name="psum_mm", bufs=2, space="PSUM") as psum_mm,
            tc.tile_pool(name="psum_s", bufs=3, space="PSUM") as psum_s,
            tc.tile_pool(name="psum_d", bufs=3, space="PSUM") as psum_d,
        ):
            # --- startup: first weights, then x, then one-hots -----------
            def load_w1(c, m, split=1):
                t = pw.tile([128, KBF, 128], dt.bfloat16, name=f"w1t_{c}_{m}", tag="w1")
                step = KBF if split == 1 else 8
                for j in range(0, KBF, step):
                    e = min(j + step, KBF)
                    nc.sync.dma_start(
                        t[:, j:e, :], w1p.ap()[m][:, j:e, :]
                    )
                t8 = pw.tile(
                    [128, NP8, 2, 128], dt.float8e4, name=f"w1t8_{c}_{m}", tag="w1f8"
                )
                nc.sync.dma_start(t8[:], w1p8.ap()[m])
                return t, t8

            # memsets first: they gate the PE warm-ups, and the gpsimd queue
            # also issues the startup x DMA triggers right after
            ones_t = psmall.tile([1, 128], dt.bfloat16, name="ones_t")
            nc.gpsimd.memset(ones_t[:], 1.0)
            eps_t = psmall.tile([1, BC], dt.bfloat16, name="eps_t")
            nc.gpsimd.memset(eps_t[:], 1e-30)

            def load_w1_head():
                # first four m-tiles, slice-interleaved so every tile's
                # low-k slices land first (the 4-way head loop walks k);
                # triggers split across three queues so serial DMA-issue
                # time (~0.65us each) never gates the PE
                tiles = []
                for m in range(4):
                    t = pw.tile(
                        [128, KBF, 128], dt.bfloat16, name=f"w1t_0_{m}", tag="w1"
                    )
                    t8 = pw.tile(
                        [128, NP8, 2, 128], dt.float8e4, name=f"w1t8_0_{m}",
                        tag="w1f8",
                    )
                    tiles.append((t, t8))
                slices = [(j, min(j + 8, KBF)) for j in range(0, KBF, 8)]
                for m in (2, 3):  # j0 slices the scalar queue issues first
                    j, e = slices[0]
                    nc.scalar.dma_start(
                        tiles[m][0][:, j:e, :], w1p.ap()[m][:, j:e, :]
                    )
                for j, e in slices:
                    for m in (0, 1):
                        nc.sync.dma_start(
                            tiles[m][0][:, j:e, :], w1p.ap()[m][:, j:e, :]
                        )
                for j, e in slices[1:]:
                    for m in (2, 3):
                        nc.gpsimd.dma_start(
                            tiles[m][0][:, j:e, :], w1p.ap()[m][:, j:e, :]
                        )
                for m in range(4):
                    eng = nc.sync if m < 2 else nc.gpsimd
                    eng.dma_start(tiles[m][1][:], w1p8.ap()[m])
                return {(0, m): tiles[m] for m in range(4)}

            def emit_xt_quads(c):
                xt = xts[c]
                quads = [(k, min(4, KBF - k)) for k in range(8, KBF, 4)]
                for i, (k, step) in enumerate(quads):
                    if k >= 12 and (i == 0 or quads[i - 1][0] < 12):
                        nc.scalar.dma_start(xt8s[c][:], xtp8.ap()[c])
                    nc.scalar.dma_start(
                        xt[:, k : k + step, :], xtp.ap()[c, :, k : k + step, :]
                    )

            def load_w2(c, m):
                t = pw.tile(
                    [128, KH, 128], dt.bfloat16, name=f"w2t_{c}_{m}", tag="w2"
                )
                nc.sync.dma_start(t[:], w2p.ap()[m])
                return t

            w2_pre = {}

            xts = {}
            xt8s = {}

            def emit_xt_load(c, pairs=None, f8=None, eng=None):
                """Load chunk c's x^T. bf16 part in k-pair/quad DMAs, fp8
                part as one DMA. eng picks the trigger queue."""
                eng = eng or nc.sync
                if c not in xts:
                    xts[c] = pbig.tile(
                        [128, KBF, BC], dt.bfloat16, name=f"xt{c}", tag="xt", bufs=2
                    )
                    xt8s[c] = pbig.tile(
                        [128, NP8, 2, BC], dt.float8e4, name=f"xt8_{c}", tag="xt8",
                        bufs=2,
                    )
                xt = xts[c]
                if pairs is None and f8 is None:
                    # startup: only the first k-pairs, on the gpsimd queue
                    # (quads follow via emit_xt_quads on the scalar queue,
                    # emitted after the w1-head j0 triggers)
                    for k in range(0, 8, 2):
                        nc.gpsimd.dma_start(
                            xt[:, k : k + 2, :], xtp.ap()[c, :, k : k + 2, :]
                        )
                    return
                ks = [(k, 2) for k in (pairs or [])]
                for k, step in ks:
                    eng.dma_start(
                        xt[:, k : k + step, :], xtp.ap()[c, :, k : k + step, :]
                    )
                if f8:
                    eng.dma_start(xt8s[c][:], xtp8.ap()[c])

            emit_xt_load(0, eng=nc.gpsimd)
            w1_pre = load_w1_head()
            emit_xt_quads(0)

            b1t = psmall.tile([128, KH], dt.float32, name="b1t")
            nc.scalar.dma_start(b1t[:], b1p.ap()[:])
            b2t = psmall.tile([128, KD], dt.float32, name="b2t")
            nc.scalar.dma_start(b2t[:], b2p.ap()[:])
            # resident one-hot tiles (single big DMA each); loaded after the
            # startup-critical w1/x stream, well before first use
            cpt_all = psmall.tile([128, NR, 128], dt.bfloat16, name="cpt_all")
            ctt_all = psmall.tile([128, NBC, 128], dt.bfloat16, name="ctt_all")
            onehot_loaded = [False]

            def load_onehots():
                if not onehot_loaded[0]:
                    nc.sync.dma_start(cpt_all[:], cpp.ap()[:])
                    nc.sync.dma_start(ctt_all[:], ctpp.ap()[:])
                    onehot_loaded[0] = True

            # PE warm-up: the first ~10us are DMA-fill bound; keep the PE
            # busy (and the HAM clock-gate warm) with throwaway matmuls that
            # depend only on the memset constants.
            warm_ps = psum_mm.tile([1, BC], dt.float32, name="warm_ps", tag="mm")

            def warm_mm(n=1):
                for _ in range(n):
                    nc.tensor.matmul(
                        warm_ps[:], ones_t[:1, :1], eps_t[:1, :], start=True, stop=True
                    )

            warm_mm(4)

            prev_tail = [None]
            dma_rr = [0]  # round-robin for output DMA trigger queues

            def out_dma(dst, src):
                eng = (nc.sync, nc.gpsimd)[dma_rr[0] & 1]
                dma_rr[0] += 1
                eng.dma_start(dst, src)

            for c in range(NB):
                cs = slice(c * BC, (c + 1) * BC)
                xt = xts[c]
                xt8 = xt8s[c]

                # ---------------- phase A: hT = W1^T @ xT (+b1) ----------
                ht = []

                def a_act(m, ps, c=c, ht=ht):
                    hm = pbig.tile(
                        [128, BC], dt.bfloat16, name=f"ht_{c}_{m}", tag=f"ht{m}"
                    )
                    nc.scalar.activation(
                        hm[:], ps[:], AF.Identity, bias=b1t[:, m : m + 1]
                    )
                    ht.append(hm)

                m_lo = 0
                if c == 0:
                    # interleave the m=0..3 k-loops: the PE consumes each
                    # arriving x pair four times, so the startup x stream
                    # never gates it. ps2/ps3 borrow the bcast psum pool,
                    # which chunk 0's phase A never touches.
                    pre = [w1_pre.pop((0, m)) for m in range(4)]
                    pss = [
                        psum_mm.tile([128, BC], dt.float32, name="psA_0_0", tag="mm"),
                        psum_mm.tile([128, BC], dt.float32, name="psA_0_1", tag="mm"),
                        psum_d.tile([128, BC], dt.float32, name="psA_0_2", tag="pd"),
                        psum_d.tile([128, BC], dt.float32, name="psA_0_3", tag="pd"),
                    ]
                    for k in range(KBF):
                        for mi in range(4):
                            nc.tensor.matmul(
                                pss[mi][:], pre[mi][0][:, k, :], xt[:, k, :],
                                start=(k == 0), stop=False,
                            )
                    for mi in range(4):
                        for p in range(NP8):
                            nc.tensor.matmul(
                                pss[mi][:], pre[mi][1][:, p, :, :],
                                xt8[:, p, :, :],
                                start=False, stop=(p == NP8 - 1), perf_mode=DR,
                            )
                        a_act(mi, pss[mi])
                    m_lo = 4
                for m in range(m_lo, KH):
                    w1t, w1t8 = w1_pre.pop((c, m), None) or load_w1(c, m)
                    ps = psum_mm.tile(
                        [128, BC], dt.float32, name=f"psA_{c}_{m}", tag="mm"
                    )
                    # bf16 k-tiles first: chunk 0's x pairs stream in k order,
                    # so the accumulation starts as soon as pair 0 lands; the
                    # small fp8 tiles have arrived long before the tail
                    for k in range(KBF):
                        nc.tensor.matmul(
                            ps[:],
                            w1t[:, k, :],
                            xt[:, k, :],
                            start=(k == 0),
                            stop=False,
                        )
                    for p in range(NP8):
                        nc.tensor.matmul(
                            ps[:],
                            w1t8[:, p, :, :],
                            xt8[:, p, :, :],
                            start=False,
                            stop=(p == NP8 - 1),
                            perf_mode=DR,
                        )
                    a_act(m, ps)
                    if m == 7:
                        # w2 for the first two decoder tiles ahead of the
                        # big one-hot DMAs, so phase B starts without a stall
                        w2_pre[(c, 0)] = load_w2(c, 0)
                        w2_pre[(c, 1)] = load_w2(c, 1)
                    if m == 10:
                        load_onehots()
                    if m == 2 and prev_tail[0] is not None:
                        # previous chunk's softmax tail: runs on the PE here,
                        # long after its recip chain finished
                        prev_tail[0]()
                        prev_tail[0] = None

                # -------- phase B: yT = W2^T @ hT (+b2), e = exp(yT) -----
                # -------- + segment reduce / recip / broadcast / out -----
                et = [None] * KD
                spsum = {}
                r_tiles = {}
                groups_done = set()
                bcast_pending = list(range(KD))
                bcast_ready = []  # ready, emission delayed one B-group

                def emit_bcast(m2, c=c, cs=cs, et=et, r_tiles=r_tiles):
                    gl = m_groups[m2]
                    pd = psum_d.tile(
                        [128, BC], dt.float32, name=f"pd_{c}_{m2}", tag="pd"
                    )
                    for idx, g in enumerate(gl):
                        n = groups[g]["nseg"]
                        nc.tensor.matmul(
                            pd[:],
                            ctt_all[:n, bc_slot[(m2, g)], :],
                            r_tiles[g][:n, :],
                            start=(idx == 0),
                            stop=(idx == len(gl) - 1),
                        )
                    ot = pev.tile([128, BC], dt.float32, name=f"ot_{c}_{m2}", tag="ot")
                    nc.vector.tensor_mul(ot[:], pd[:], et[m2][:])
                    out_dma(outp.ap()[m2][:, cs], ot[:])

                def flush_bcast(
                    max_n=None, bcast_ready=bcast_ready, emit_bcast=emit_bcast
                ):
                    n = len(bcast_ready) if max_n is None else max_n
                    for m2 in bcast_ready[:n]:
                        emit_bcast(m2)
                    del bcast_ready[:n]

                def emit_reduce(
                    k,
                    c=c,
                    et=et,
                    spsum=spsum,
                    r_tiles=r_tiles,
                    groups_done=groups_done,
                    bcast_pending=bcast_pending,
                    bcast_ready=bcast_ready,
                ):
                    for g, slot, is_first, is_last in red_by_k[k]:
                        n = groups[g]["nseg"]
                        if is_first:
                            spsum[g] = psum_s.tile(
                                [128, BC], dt.float32, name=f"pss_{c}_{g}", tag="ps_s"
                            )
                        nc.tensor.matmul(
                            spsum[g][:n, :],
                            cpt_all[:, slot, :n],
                            et[k][:],
                            start=is_first,
                            stop=is_last,
                        )
                        if is_last:
                            # 1/s as exp(-ln(s)) on the ACT engine: both live
                            # in the natural_log_exp_and_others table (with
                            # Exp/Identity), so no table reloads, and the DVE
                            # iterative-divide (3.4us) leaves the chain
                            ls = pr32.tile(
                                [128, BC], dt.float32, name=f"ls_{c}_{g}", tag="r32"
                            )
                            nc.scalar.activation(ls[:n, :], spsum[g][:n, :], AF.Ln)
                            rg = pbig.tile(
                                [128, BC], dt.bfloat16, name=f"r_{c}_{g}",
                                tag=f"r{g % 6}",
                            )
                            nc.scalar.activation(rg[:n, :], ls[:n, :], AF.Exp, scale=-1.0)
                            r_tiles[g] = rg
                            groups_done.add(g)
                            # queue feature tiles whose groups are all ready
                            still = []
                            for m2 in bcast_pending:
                                if et[m2] is not None and all(
                                    gg in groups_done for gg in m_groups[m2]
                                ):
                                    bcast_ready.append(m2)
                                else:
                                    still.append(m2)
                            bcast_pending[:] = still

                for m in range(KD):
                    if c + 1 < NB and m < KBF // 2:
                        # trickle next chunk's x prefetch: one k-pair per
                        # B-group so it never bursts against the W2 stream
                        emit_xt_load(c + 1, pairs=[2 * m], f8=(m == KBF // 2 - 1))
                    w2t = w2_pre.pop((c, m), None) or load_w2(c, m)
                    ps = psum_mm.tile(
                        [128, BC], dt.float32, name=f"psB_{c}_{m}", tag="mm"
                    )
                    for k in range(KH):
                        nc.tensor.matmul(
                            ps[:],
                            w2t[:, k, :],
                            ht[k][:],
                            start=(k == 0),
                            stop=(k == KH - 1),
                        )
                    em = pbig.tile(
                        [128, BC], dt.bfloat16, name=f"et_{c}_{m}", tag=f"et{m}"
                    )
                    nc.scalar.activation(em[:], ps[:], AF.Exp, bias=b2t[:, m : m + 1])
                    et[m] = em
                    # delayed work: bcasts queued >=1 B-group ago (trickled
                    # so DVE mult bursts never delay a reciprocal), then the
                    # reduce for k-tile m-1 (the lag hides ACT/DVE latency)
                    flush_bcast(max_n=3)
                    if m >= 1:
                        emit_reduce(m - 1)

                # tail part 1 now: the final reduce + recip chain starts
                # immediately after the last B group
                emit_reduce(KD - 1)

                def tail(
                    flush_bcast=flush_bcast,
                    emit_bcast=emit_bcast,
                    bcast_pending=bcast_pending,
                ):
                    flush_bcast()
                    for m2 in bcast_pending:
                        emit_bcast(m2)
                    bcast_pending.clear()

                if c + 1 < NB:
                    # defer part 2: the PE executes the remaining broadcasts
                    # inside the next chunk's phase A, by which time the
                    # recip chain is long done
                    prev_tail[0] = tail
                else:
                    tail()

    _split_excess_waits(nc)
    return nc


def _pack_inputs(x, segment_ids, W1, b1, W2, b2):
    """Host-side shard + pack. Returns in_maps (one dict per core)."""
    import ml_dtypes

    bf16 = ml_dtypes.bfloat16
    f8 = ml_dtypes.float8_e4m3
    seg = np.asarray(segment_ids)
    groups, red, red_by_k, bc, m_groups, bc_slot = _segment_plan(seg)
    NR, NBC = len(red), len(bc)
    seg64 = seg.astype(np.int64)

    # one-hot tiles for the segment matmuls (partition-major packing)
    cp = np.zeros((NR, 128, 128), dtype=bf16)
    for i, (k, gi) in enumerate(red):
        g = groups[gi]
        loc = seg64[k * 128 : (k + 1) * 128] - g["base"]
        rows = np.arange(128)
        mask = (loc >= 0) & (loc < g["nseg"]) & (
            (seg64[k * 128 : (k + 1) * 128] >= g["base"])
            & (seg64[k * 128 : (k + 1) * 128] < g["end"])
        )
        cp[i, rows[mask], loc[mask]] = 1
    # fake entries for empty segments: one arbitrary feature row in the
    # group's first reduce tile keeps the segment sum positive; the bcast
    # one-hot column for an empty segment is all-zero so outputs are clean.
    present = np.zeros(S, dtype=bool)
    present[seg64] = True
    first_slot = {}
    for i, (k, gi) in enumerate(red):
        if gi not in first_slot:
            first_slot[gi] = i
    for gi, g in enumerate(groups):
        for s in range(g["base"], g["end"]):
            if not present[s]:
                cp[first_slot[gi], 0, s - g["base"]] = 1

    ctp = np.zeros((NBC, 128, 128), dtype=bf16)
    for j, (m, gi) in enumerate(bc):
        g = groups[gi]
        svals = seg64[m * 128 : (m + 1) * 128]
        loc = svals - g["base"]
        cols = np.arange(128)
        mask = (svals >= g["base"]) & (svals < g["end"])
        ctp[j, loc[mask], cols[mask]] = 1
    cpp = np.ascontiguousarray(cp.transpose(1, 0, 2))
    ctpp = np.ascontiguousarray(ctp.transpose(1, 0, 2))

    W1bf = W1[: KBF * 128]
    W1f8 = W1[KBF * 128 :]
    w1p = np.ascontiguousarray(
        W1bf.reshape(KBF, 128, KH, 128).transpose(2, 1, 0, 3)
    ).astype(bf16)
    w1p8 = np.ascontiguousarray(
        W1f8.reshape(NP8, 2, 128, KH, 128).transpose(3, 2, 0, 1, 4)
    ).astype(f8)
    w2p = np.ascontiguousarray(
        W2.reshape(KH, 128, KD, 128).transpose(2, 1, 0, 3)
    ).astype(bf16)
    b1p = np.ascontiguousarray(b1.reshape(KH, 128).T).astype(np.float32)
    b2p = np.ascontiguousarray(b2.reshape(KD, 128).T).astype(np.float32)

    in_maps = []
    for core in range(NCORES):
        xs = x[core * BS : (core + 1) * BS]  # [BS, D]
        xbf = xs[:, : KBF * 128]
        xf8 = xs[:, KBF * 128 :]
        xtp = np.ascontiguousarray(
            xbf.reshape(NB, BC, KBF, 128).transpose(0, 3, 2, 1)
        ).astype(bf16)
        xtp8 = np.ascontiguousarray(
            xf8.reshape(NB, BC, NP8, 2, 128).transpose(0, 4, 2, 3, 1)
        ).astype(f8)
        in_maps.append(
            {
                "xtp": xtp,
                "xtp8": xtp8,
                "w1p": w1p,
                "w1p8": w1p8,
                "w2p": w2p,
                "b1p": b1p,
                "b2p": b2p,
                "cpp": cpp,
                "ctpp": ctpp,
            }
        )
    return in_maps


def _unpack_outputs(results):
    """results: list (per core) of {"outp": [KD, 128, BS]} -> [B, D] f32."""
    parts = []
    for core in range(NCORES):
        outp = results[core]["outp"]  # [KD, 128, BS]
        parts.append(np.ascontiguousarray(outp.transpose(2, 0, 1)).reshape(BS, D))
    return np.concatenate(parts, axis=0)


_CACHE = {}

# test harness hooks (not used in the graded path)
TRACE = False
TRACE_ALL_CORES = False
LAST_RESULT = None


def kernel(x, segment_ids, W1, b1, W2, b2):
    global LAST_RESULT
    _import_concourse()
    from concourse.bass_utils import run_bass_kernel_spmd

    key = np.asarray(segment_ids).tobytes()
    if key not in _CACHE:
        _CACHE[key] = _build_program(segment_ids)
    nc = _CACHE[key]

    in_maps = _pack_inputs(
        np.asarray(x, dtype=np.float32),
        segment_ids,
        np.asarray(W1, dtype=np.float32),
        np.asarray(b1, dtype=np.float32),
        np.asarray(W2, dtype=np.float32),
        np.asarray(b2, dtype=np.float32),
    )
    kw = {"trace_cores": list(range(NCORES))} if TRACE_ALL_CORES else {}
    res = run_bass_kernel_spmd(nc, in_maps, list(range(NCORES)), trace=TRACE, **kw)
    LAST_RESULT = res
    return _unpack_outputs(res.results)
